# revision 1
# baseline (speedup 1.0000x reference)
"""BondEncoder Trainium2 kernel: GRU over atoms + neighbor-gather + FC + leaky relu.

Data-parallel over batch: 8 cores x 32 molecules each. No collectives.

Per-core pipeline:
  1. cast-load x (f32->bf16), DMA-transpose to x^T [IN, (b,n)]
  2. x_proj = W_ih^T.T @ x^T  (+ biases folded during PSUM evacuation)
  3. GRU recurrence, 128 serial steps: W_hh^T stationary (bf16, FWL), gate
     elementwise chain balanced across DVE/ACT/GpSimd
  4. FC per molecule: Y_v = rnn_b @ W_v^T (rnn chunks stationary), evacuate
     Y to SBUF, then out_b = sum_v P_vb^T @ Y_v with host-built one-hot
     selection matrices P as stationary operands (gather-as-matmul; the
     runtime lacks HIPI ucode for dma_gather), +b_fc via K=1 ones matmul,
     leaky-relu = max(x, 0.1x), row-contiguous DMA out
"""

import sys

sys.path.insert(0, "/opt/trn_rl_repo")

import numpy as np
import ml_dtypes

import concourse.bacc as bacc
import concourse.bass as bass
import concourse.mybir as mybir
from concourse import tile
from concourse.bass_utils import run_bass_kernel_spmd

BF16 = mybir.dt.bfloat16
F32 = mybir.dt.float32
ADD = mybir.AluOpType.add
MULT = mybir.AluOpType.mult
MAX = mybir.AluOpType.max
AF = mybir.ActivationFunctionType

NCORES = 8
B, T, IN, H, V, OUT = 256, 128, 128, 256, 6, 256
G3 = 3 * H  # 768
BC = B // NCORES  # 32 molecules per core
NT = BC * T  # 4096 tokens per core
NGRP = 1  # recurrence batch groups
GB = BC // NGRP  # molecules per group


def build_program(
    debug: bool = False, rec_steps: int = T, do_fc: bool = True, loop_n: int = 0
) -> bacc.Bacc:
    nc = bacc.Bacc("TRN2", target_bir_lowering=False, debug=debug)

    x_d = nc.declare_dram_parameter("x", [BC, T, IN], F32, isOutput=False)
    w_ihT_d = nc.declare_dram_parameter("w_ihT", [IN, G3], BF16, isOutput=False)
    w_hhT_d = nc.declare_dram_parameter("w_hhT", [2, 128, G3], BF16, isOutput=False)
    w_fcT_d = nc.declare_dram_parameter("w_fcT", [12, 128, OUT], BF16, isOutput=False)
    bias_ev_d = nc.declare_dram_parameter("bias_ev", [128, 6], F32, isOutput=False)
    b_hhn_d = nc.declare_dram_parameter("b_hhn", [128, 2, GB], F32, isOutput=False)
    b_fc_d = nc.declare_dram_parameter("b_fc", [1, OUT], BF16, isOutput=False)
    p_d = nc.declare_dram_parameter("ponehot", [BC, V, 128, 128], BF16, isOutput=False)
    out_d = nc.declare_dram_parameter("out", [NT, OUT], F32, isOutput=True)

    from contextlib import nullcontext

    with tile.TileContext(nc) as tc:
        with tc.For_i(0, loop_n, 1) if loop_n > 0 else nullcontext():
          with (
            tc.tile_pool(name="const", bufs=1) as cp,
            tc.tile_pool(name="rnn", bufs=1) as rp,
          ):
              # ---- constants ----
              w_ihT = cp.tile([IN, G3], BF16, tag="w_ihT")
              nc.sync.dma_start(w_ihT[:], w_ihT_d[:])
              w_hhT = cp.tile([128, 2, G3], BF16, tag="w_hhT")
              nc.sync.dma_start(w_hhT[:], w_hhT_d[:].rearrange("k p g -> p k g"))
              w_fcT = cp.tile([128, 12, OUT], BF16, tag="w_fcT")
              nc.sync.dma_start(w_fcT[:], w_fcT_d[:].rearrange("k p o -> p k o"))
              bias_ev = cp.tile([128, 6], F32, tag="bias_ev")
              nc.sync.dma_start(bias_ev[:], bias_ev_d[:])
              b_hhn = cp.tile([128, 2, GB], F32, tag="b_hhn")
              nc.sync.dma_start(b_hhn[:], b_hhn_d[:])
              b_fc = cp.tile([1, OUT], BF16, tag="b_fc")
              nc.sync.dma_start(b_fc[:], b_fc_d[:])
              ones = cp.tile([1, 128], BF16, tag="ones")
              nc.vector.memset(ones[:], 1.0)

              # rnn hidden states, feature-major: [feat%128, chunk, molecule, step]
              rnn_fp = rp.tile([128, 2, BC, T], BF16, tag="rnn_fp")

              with tc.tile_pool(name="xio", bufs=1) as xiop:
                  # ---- x: cast-load then transpose to [IN, (b,n)] ----
                  x_sb = xiop.tile([128, BC, IN], BF16, tag="x_sb")
                  nc.gpsimd.dma_start(x_sb[:], x_d[:].rearrange("b n i -> n b i"))
                  xT = xiop.tile([IN, BC, T], BF16, tag="xT")
                  for b in range(BC):
                      eng = nc.sync if b % 2 == 0 else nc.scalar
                      eng.dma_start_transpose(xT[:, b, :], x_sb[:, b, :])

                  with tc.tile_pool(name="xp", bufs=1) as xpp:
                      # xp layout: [part, step, group, gatecol] gatecol=(m,16b)
                      xp = xpp.tile([128, T, NGRP, 6 * GB], BF16, tag="xp")
                      with tc.tile_pool(name="xppsum", bufs=2, space="PSUM") as xpps:
                          for m in range(6):
                              for cq in range(2):
                                  ps = xpps.tile([128, 16, 128], F32, tag="xpps")
                                  for cc in range(4):
                                      c = cq * 4 + cc
                                      nc.tensor.matmul(
                                          ps[:, cc * 4 : (cc + 1) * 4, :],
                                          w_ihT[:, m * 128 : (m + 1) * 128],
                                          xT[:, c * 4 : (c + 1) * 4, :],
                                          start=True,
                                          stop=True,
                                      )
                                  o = m * GB + cq * 16
                                  dst = xp[:, :, 0, o : o + 16].rearrange(
                                      "p t d -> p d t"
                                  )
                                  bias = bias_ev[:, m : m + 1]
                                  if (m * 2 + cq) % 2 == 0:
                                      nc.vector.tensor_scalar(
                                          dst, ps[:], bias, None, ADD
                                      )
                                  else:
                                      nc.scalar.activation(
                                          dst, ps[:], AF.Identity, bias=bias
                                      )

                      # ---- GRU recurrence ----
                      with (
                          tc.tile_pool(name="rec", bufs=3) as rcp,
                          tc.tile_pool(name="hp", bufs=3) as hp,
                          tc.tile_pool(name="ghps", bufs=2, space="PSUM") as ghp,
                      ):
                          h_cur = []
                          for g in range(NGRP):
                              hz = hp.tile([128, 2 * GB], BF16, tag=f"h{g}")
                              nc.vector.memset(hz[:], 0.0)
                              h_cur.append(hz)

                          for t in range(rec_steps):
                              for g in range(NGRP):
                                  h_prev = h_cur[g]
                                  gha = ghp.tile([128, 4, GB], F32, tag=f"gha{g}")
                                  ghb = ghp.tile([128, 2, GB], F32, tag=f"ghb{g}")
                                  # k-outer: K0 matmuls need only h chunk 0.
                                  # start=True only on the first MM per PSUM
                                  # bank (clears has_written once); first
                                  # writes then overwrite, k=1 accumulates.
                                  for k in range(2):
                                      for m in range(6):
                                          dst = (
                                              gha[:, m, :]
                                              if m < 4
                                              else ghb[:, m - 4, :]
                                          )
                                          nc.tensor.matmul(
                                              dst,
                                              w_hhT[:, k, m * 128 : (m + 1) * 128],
                                              h_prev[:, k * GB : (k + 1) * GB],
                                              start=(k == 0 and m in (0, 4)),
                                              stop=(k == 1 and m in (3, 5)),
                                              skip_group_check=True,
                                          )
                                  xpt = xp[:, t, g, :]
                                  ur = ghp.tile([128, 2, GB], F32, tag=f"ur{g}")
                                  nc.vector.tensor_tensor(
                                      ur[:],
                                      gha[:, 0:2, :],
                                      xpt[:, 0 : 2 * GB].rearrange(
                                          "p (m b) -> p m b", m=2
                                      ),
                                      ADD,
                                  )
                                  r_sb = rcp.tile([128, 2, GB], BF16, tag=f"r{g}")
                                  nc.scalar.activation(r_sb[:], ur[:], AF.Sigmoid)
                                  uz = ghp.tile([128, 2, GB], F32, tag=f"uz{g}")
                                  nc.vector.tensor_tensor(
                                      uz[:],
                                      gha[:, 2:4, :],
                                      xpt[:, 2 * GB : 4 * GB].rearrange(
                                          "p (m b) -> p m b", m=2
                                      ),
                                      ADD,
                                  )
                                  z_sb = rcp.tile([128, 2, GB], BF16, tag=f"z{g}")
                                  nc.scalar.activation(z_sb[:], uz[:], AF.Sigmoid)
                                  t1a = rcp.tile([128, 2, GB], BF16, tag=f"t1a{g}")
                                  nc.vector.tensor_tensor(
                                      t1a[:], ghb[:], b_hhn[:], ADD
                                  )
                                  t1 = rcp.tile([128, 2, GB], BF16, tag=f"t1{g}")
                                  nc.vector.tensor_tensor(
                                      t1[:], t1a[:], r_sb[:], MULT
                                  )
                                  t2 = rcp.tile([128, 2, GB], BF16, tag=f"t2{g}")
                                  nc.vector.tensor_tensor(
                                      t2[:],
                                      t1[:],
                                      xpt[:, 4 * GB : 6 * GB].rearrange(
                                          "p (c b) -> p c b", c=2
                                      ),
                                      ADD,
                                  )
                                  q_ = rcp.tile([128, 2, GB], BF16, tag=f"q{g}")
                                  nc.vector.tensor_tensor(
                                      q_[:],
                                      z_sb[:],
                                      h_prev[:].rearrange("p (c b) -> p c b", c=2),
                                      MULT,
                                  )
                                  u1 = rcp.tile([128, 2, GB], BF16, tag=f"u1{g}")
                                  nc.vector.tensor_scalar(
                                      u1[:], z_sb[:], -1.0, 1.0, MULT, ADD
                                  )
                                  nn_ = rcp.tile([128, 2, GB], BF16, tag=f"nn{g}")
                                  h_new = hp.tile([128, 2 * GB], BF16, tag=f"h{g}")
                                  # per-chunk tail: h chunk 0 completes early so
                                  # the next step's K0 matmuls can begin
                                  for c in range(2):
                                      nc.scalar.activation(
                                          nn_[:, c, :], t2[:, c, :], AF.Tanh
                                      )
                                      mc = rcp.tile([128, GB], BF16, tag=f"m{g}{c}")
                                      nc.vector.tensor_tensor(
                                          mc[:], nn_[:, c, :], u1[:, c, :], MULT
                                      )
                                      nc.vector.tensor_tensor(
                                          h_new[:, c * GB : (c + 1) * GB],
                                          mc[:],
                                          q_[:, c, :],
                                          ADD,
                                      )
                                  nc.gpsimd.tensor_copy(
                                      rnn_fp[:, :, g * GB : (g + 1) * GB, t],
                                      h_new[:].rearrange("p (c b) -> p c b", c=2),
                                  )
                                  h_cur[g] = h_new

              # ---- FC: Y = rnn @ W_v^T ; out = P^T Y (one-hot gather) ----
              if not do_fc:
                  osb0 = rp.tile([128, OUT], F32, tag="osb0")
                  nc.vector.tensor_copy(osb0[:], rnn_fp[:, 0, 0:2, :])
                  nc.sync.dma_start(out_d[0:128, :], osb0[:])
              else:
                  with (
                      tc.tile_pool(name="pone", bufs=8) as pop,
                      tc.tile_pool(name="fc", bufs=4) as fcp,
                      tc.tile_pool(name="yps", bufs=2, space="PSUM") as yps,
                      tc.tile_pool(name="fcps", bufs=2, space="PSUM") as fcps,
                  ):
                      for b in range(BC):
                          p_sb = pop.tile([128, V, 128], BF16, tag="p_sb")
                          nc.sync.dma_start(
                              p_sb[:],
                              p_d[b].rearrange("v p d -> p v d"),
                          )
                          yp = yps.tile([128, V, OUT], F32, tag="yp")
                          for v in range(V):
                              for c in range(2):
                                  nc.tensor.matmul(
                                      yp[:, v, :],
                                      rnn_fp[:, c, b, :],
                                      w_fcT[:, v * 2 + c, :],
                                      start=(c == 0),
                                      stop=(c == 1),
                                  )
                          ysb = fcp.tile([128, V, OUT], BF16, tag="ysb")
                          nc.vector.tensor_copy(ysb[:, 0:3, :], yp[:, 0:3, :])
                          nc.scalar.copy(ysb[:, 3:6, :], yp[:, 3:6, :])
                          ps = fcps.tile([128, OUT], F32, tag="fcps")
                          for v in range(V):
                              nc.tensor.matmul(
                                  ps[:],
                                  p_sb[:, v, :],
                                  ysb[:, v, :],
                                  start=(v == 0),
                                  stop=False,
                              )
                          nc.tensor.matmul(
                              ps[:], ones[:], b_fc[:], start=False, stop=True
                          )
                          lsb = fcp.tile([128, OUT], F32, tag="lsb")
                          nc.scalar.mul(lsb[:], ps[:], 0.1)
                          osb = fcp.tile([128, OUT], F32, tag="osb")
                          nc.vector.tensor_tensor(osb[:], ps[:], lsb[:], MAX)
                          nc.sync.dma_start(
                              out_d[b * 128 : (b + 1) * 128, :], osb[:]
                          )

    nc.compile()
    return nc


def prep_core_inputs(inputs: dict) -> list[dict]:
    """Shard + lay out inputs for each of the 8 cores (numpy, layout only)."""
    x = np.ascontiguousarray(np.asarray(inputs["x"], dtype=np.float32))
    bonded = np.asarray(inputs["bonded_atoms"]).astype(np.int64)
    w_ih = np.asarray(inputs["W_ih"], dtype=np.float32)
    w_hh = np.asarray(inputs["W_hh"], dtype=np.float32)
    b_ih = np.asarray(inputs["b_ih"], dtype=np.float32)
    b_hh = np.asarray(inputs["b_hh"], dtype=np.float32)
    w_fc = np.asarray(inputs["W_fc"], dtype=np.float32)
    b_fc = np.asarray(inputs["b_fc"], dtype=np.float32)

    bf = ml_dtypes.bfloat16
    w_ihT = np.ascontiguousarray(w_ih.T).astype(bf)  # [IN, 3H]
    w_hhT_full = w_hh.T  # [H, 3H]
    w_hhT = np.ascontiguousarray(
        w_hhT_full.reshape(2, 128, G3)
    ).astype(bf)  # [2,128,3H]
    w_fcT = np.ascontiguousarray(
        w_fc.reshape(OUT, 12, 128).transpose(1, 2, 0)
    ).astype(bf)  # [12,128,OUT]

    bb = b_ih + b_hh
    cols = [bb[m * 128 : (m + 1) * 128] for m in range(4)] + [
        b_ih[512 + m * 128 : 512 + (m + 1) * 128] for m in range(2)
    ]
    bias_ev = np.ascontiguousarray(np.stack(cols, axis=1)).astype(np.float32)
    b_hhn = np.ascontiguousarray(
        np.broadcast_to(
            np.stack([b_hh[512:640], b_hh[640:768]], axis=1)[:, :, None],
            (128, 2, GB),
        )
    ).astype(np.float32)
    b_fc_r = np.ascontiguousarray(b_fc.reshape(1, OUT)).astype(bf)

    in_maps = []
    for core in range(NCORES):
        bs = slice(core * BC, (core + 1) * BC)
        xc = np.ascontiguousarray(x[bs])
        bd = bonded[bs]  # [BC, T, V]
        # one-hot selection matrices: P[b,v,s,d] = 1 iff bonded[b,d,v] == s
        P = np.zeros((BC, V, 128, 128), dtype=bf)
        dd = np.arange(T)
        for b in range(BC):
            for v in range(V):
                P[b, v, bd[b, :, v], dd] = 1
        in_maps.append(
            {
                "x": xc,
                "w_ihT": w_ihT,
                "w_hhT": w_hhT,
                "w_fcT": w_fcT,
                "bias_ev": bias_ev,
                "b_hhn": b_hhn,
                "b_fc": b_fc_r,
                "ponehot": P,
            }
        )
    return in_maps


_NC_CACHE: dict = {}


def _get_program():
    if "nc" not in _NC_CACHE:
        _NC_CACHE["nc"] = build_program()
    return _NC_CACHE["nc"]


def kernel(**inputs) -> np.ndarray:
    nc = _get_program()
    in_maps = prep_core_inputs(inputs)
    res = run_bass_kernel_spmd(nc, in_maps, core_ids=list(range(NCORES)))
    outs = [
        np.asarray(res.results[i]["out"], dtype=np.float32).reshape(BC, T, OUT)
        for i in range(NCORES)
    ]
    return np.concatenate(outs, axis=0)


if __name__ == "__main__":
    prog = build_program()
    print("program built+compiled OK")

